# revision 1
# baseline (speedup 1.0000x reference)
"""CTC greedy decode kernel for Trainium2 (Bass/Tile), 8-core data-parallel.

Problem: log_probs [32, 4096, 1025] f32, input_lengths [32] i64 ->
  preds    [32, 4096] int32  (per-frame argmax)
  keep     [32, 4096] bool   (non-blank & != prev & t < len)
  max_logp [32, 4096] f32    (value at argmax)

Sharding: batch dim across 8 cores (4 utterances each). Per core:
16384 frames x 1025 vocab, viewed as a [128, 128] grid of frame-tiles
with frame f = p*128 + c (p = SBUF partition, c = grid column). Each
utterance owns 32 partition rows and time runs along the FREE dim:
t = (p%32)*128 + c. That makes the CTC "previous frame" a plain
shifted-AP compare (idxf[:, c] vs idxf[:, c-1]) -- no partition-shift
DMA, no DMA-semaphore hop on the critical tail. Only column 0 wraps
(prev of frame (p%32)*128 is (p-1, 127)), staged via one tiny DMA that
overlaps the last column's compute by processing columns in the order
1..127, then 0.

Argmax without a second DVE scan (the two-pass reduce+max_index version
is vector-bound at ~273us; DMA of the 67MB/core input is ~187us):

  1. DVE tensor_tensor_scan (op0=op1=max, data1=data0) computes the
     running prefix-max P_v of each frame in ONE pass. Its last element
     is the frame max m (exact f32, also the max_logp output).
  2. The Activation engine computes Sign(-P_v + m) -- 1 where P_v < m,
     0 where P_v == m -- and its accum_out sums the pass: the count of
     prefix positions strictly below the max IS the argmax index, with
     exact first-occurrence tie-breaking (jnp.argmax semantics) for any
     input, duplicates included.

So DVE does one 1.04ns/elem pass (~150us), ACT one 0.83ns/elem pass
(~160us incl per-inst SBUF access), and the ~188us HBM stream is the
critical path. The last 19 columns load per-column so DVE/ACT drain
their one-group pipeline lag before the stream ends; the post-stream
tail is one column's load-sem+scan+sign plus a couple of [128,1] mask
ops and the final store.
"""

import numpy as np

import concourse.bacc as bacc
import concourse.mybir as mybir
from concourse.tile import TileContext
from concourse.bass_utils import run_bass_kernel_spmd

B, T, V = 32, 4096, 1025
BLANK = 1024
NCORES = 8
BLOC = B // NCORES        # utterances per core
F = BLOC * T              # frames per core
P = 128                   # partitions
NT = F // P               # grid columns (128)
RPU = P // BLOC           # partition rows per utterance (32)
NGRP = 27                 # 4-col groups covering columns 1..108
PERCOL0 = 109             # columns 109..127 load per-column, col 0 last

_CACHE = {}


def _build_program():
    nc = bacc.Bacc(None, target_bir_lowering=False)
    f32 = mybir.dt.float32
    i32 = mybir.dt.int32
    lp = nc.dram_tensor("lp", [F, V], f32, kind="ExternalInput")
    valid = nc.dram_tensor("valid", [P, NT], f32, kind="ExternalInput")
    preds_o = nc.dram_tensor("preds", [P, NT], i32, kind="ExternalOutput")
    keep_o = nc.dram_tensor("keep", [P, NT], i32, kind="ExternalOutput")
    mlp_o = nc.dram_tensor("maxlp", [P, NT], f32, kind="ExternalOutput")

    # frame f = p*128 + c  ->  [p, c, v]; a 4-column tile is a contiguous
    # 16400B run per partition in HBM
    lp_r = lp.rearrange("(p n) v -> p n v", n=NT)
    SIGN = mybir.ActivationFunctionType.Sign

    with TileContext(nc) as tc:
        with (
            tc.tile_pool(name="loads", bufs=4) as loads,
            tc.tile_pool(name="tails", bufs=12) as tails,
            tc.tile_pool(name="pms", bufs=3) as pms,
            tc.tile_pool(name="pmts", bufs=6) as pmts,
            tc.tile_pool(name="sgs", bufs=4) as sgs,
            tc.tile_pool(name="persist", bufs=1) as pp,
        ):
            first = loads.tile([P, 4, V], f32, tag="big")
            nc.sync.dma_start(out=first[:], in_=lp_r[:, 1:5, :])

            idxf = pp.tile([P, NT], f32)     # argmax index (exact int in f32)
            gmax = pp.tile([P, NT], f32)     # frame max (max_logp output)
            validt = pp.tile([P, NT], f32)
            wrap = pp.tile([P, 1], f32)      # prev for column 0
            k1 = pp.tile([P, NT], f32)
            kp = pp.tile([P, NT], f32)
            preds_i = pp.tile([P, NT], i32)
            keep_i = pp.tile([P, NT], i32)

            nc.sync.dma_start(out=validt[:], in_=valid[:])

            def tile_pass(src2d, pm2d, col):
                # one frame-column: prefix-max scan, then Sign+accumulate
                nc.vector.tensor_tensor_scan(
                    out=pm2d, data0=src2d, data1=src2d,
                    initial=-3.0e38,
                    op0=mybir.AluOpType.max, op1=mybir.AluOpType.max,
                )
                sg = sgs.tile([P, V], f32, tag="sg")
                nc.scalar.activation(
                    out=sg[:], in_=pm2d, func=SIGN,
                    bias=pm2d[:, V - 1 : V], scale=-1.0,
                    accum_out=idxf[:, col : col + 1],
                )

            def percol(c):
                bt = tails.tile([P, 1, V], f32, tag="tail")
                nc.sync.dma_start(out=bt[:], in_=lp_r[:, c : c + 1, :])
                pmt = pmts.tile([P, 1, V], f32, tag="pmt")
                tile_pass(bt[:, 0, :], pmt[:, 0, :], c)
                nc.gpsimd.tensor_copy(
                    out=gmax[:, c : c + 1], in_=pmt[:, 0, V - 1 : V]
                )

            # columns 1..108 in 4-column tiles
            for g in range(NGRP):
                c0 = 1 + g * 4
                if g == 0:
                    big = first
                else:
                    big = loads.tile([P, 4, V], f32, tag="big")
                    nc.sync.dma_start(out=big[:], in_=lp_r[:, c0 : c0 + 4, :])
                pm = pms.tile([P, 4, V], f32, tag="pm")
                for i in range(4):
                    tile_pass(big[:, i, :], pm[:, i, :], c0 + i)
                nc.gpsimd.tensor_copy(
                    out=gmax[:, c0 : c0 + 4], in_=pm[:, :, V - 1]
                )

            # columns 109..127 per-column (lets DVE/ACT drain their lag).
            # Column 127 runs five slots early so the wrap staging below has
            # its data (and its DMA-sem latency) retired well before the
            # final column's compute needs it.
            order = list(range(PERCOL0, NT - 1))
            order = order[:13] + [NT - 1] + order[13:]
            for c in order:
                percol(c)

            # column 0 last, split into two half-vocab loads with a chained
            # prefix-max so the final scan work after the stream's last
            # byte is one half, not a full column
            HV = 513
            bt0 = tails.tile([P, 1, V], f32, tag="tail")
            nc.sync.dma_start(out=bt0[:, :, 0:HV], in_=lp_r[:, 0:1, 0:HV])
            nc.sync.dma_start(out=bt0[:, :, HV:V], in_=lp_r[:, 0:1, HV:V])

            # stage prev-of-column-0: wrap[p] = idxf[p-1, 127]; rows at
            # utterance starts become the -1 sentinel. Emitted after every
            # load so its ACT_127 sem-wait can't block the SP load stream;
            # col 127 ran early, so the staging retires well before kp0.
            nc.sync.dma_start(
                out=wrap[1:P, :], in_=idxf[0 : P - 1, NT - 1 : NT]
            )
            for u in range(BLOC):
                nc.gpsimd.memset(wrap[u * RPU : u * RPU + 1, :], -1.0)
            pmt0 = pmts.tile([P, 1, V], f32, tag="pmt")
            nc.vector.tensor_tensor_scan(
                out=pmt0[:, 0, 0:HV], data0=bt0[:, 0, 0:HV],
                data1=bt0[:, 0, 0:HV], initial=-3.0e38,
                op0=mybir.AluOpType.max, op1=mybir.AluOpType.max,
            )
            nc.vector.tensor_tensor_scan(
                out=pmt0[:, 0, HV:V], data0=bt0[:, 0, HV:V],
                data1=bt0[:, 0, HV:V], initial=pmt0[:, 0, HV - 1 : HV],
                op0=mybir.AluOpType.max, op1=mybir.AluOpType.max,
            )

            # column 0's index via ACT sign+accum: the ACT engine is idle
            # after column 126, and this frees DVE to run the [1:NT] mask
            # ops concurrently with column 0's sign pass
            sg0 = sgs.tile([P, V], f32, tag="sg")
            nc.scalar.activation(
                out=sg0[:], in_=pmt0[:, 0, :], func=SIGN,
                bias=pmt0[:, 0, V - 1 : V], scale=-1.0,
                accum_out=idxf[:, 0:1],
            )
            nc.gpsimd.tensor_copy(
                out=gmax[:, 0:1], in_=pmt0[:, 0, V - 1 : V]
            )

            # epilogue: full-width masks and exactly three output stores.
            # Fewer, bigger stores beat split early/late stores -- each
            # extra DMA costs ~700ns of serialized HWDGE at the very end,
            # more than the ~1.2us by which column 0 trails the rest.
            nc.sync.dma_start(out=mlp_o[:], in_=gmax[:])
            nc.vector.tensor_copy(out=preds_i[:, 1:NT], in_=idxf[:, 1:NT])
            nc.vector.tensor_scalar(
                out=k1[:, 1:NT], in0=idxf[:, 1:NT],
                scalar1=float(BLANK), scalar2=None,
                op0=mybir.AluOpType.not_equal,
            )
            nc.vector.tensor_tensor(
                out=k1[:, 1:NT], in0=k1[:, 1:NT], in1=validt[:, 1:NT],
                op=mybir.AluOpType.mult,
            )
            nc.vector.tensor_tensor(
                out=kp[:, 1:NT], in0=idxf[:, 1:NT], in1=idxf[:, 0 : NT - 1],
                op=mybir.AluOpType.not_equal,
            )
            nc.vector.tensor_tensor(
                out=keep_i[:, 1:NT], in0=kp[:, 1:NT], in1=k1[:, 1:NT],
                op=mybir.AluOpType.mult,
            )
            nc.vector.tensor_copy(out=preds_i[:, 0:1], in_=idxf[:, 0:1])
            nc.vector.tensor_scalar(
                out=k1[:, 0:1], in0=idxf[:, 0:1],
                scalar1=float(BLANK), scalar2=validt[:, 0:1],
                op0=mybir.AluOpType.not_equal, op1=mybir.AluOpType.mult,
            )
            nc.vector.tensor_tensor(
                out=kp[:, 0:1], in0=idxf[:, 0:1], in1=wrap[:],
                op=mybir.AluOpType.not_equal,
            )
            nc.vector.tensor_tensor(
                out=keep_i[:, 0:1], in0=kp[:, 0:1], in1=k1[:, 0:1],
                op=mybir.AluOpType.mult,
            )
            nc.sync.dma_start(out=preds_o[:], in_=preds_i[:])
            nc.sync.dma_start(out=keep_o[:], in_=keep_i[:])
    nc.compile()
    return nc


def _host_inputs(log_probs, input_lengths):
    log_probs = np.ascontiguousarray(np.asarray(log_probs, dtype=np.float32))
    lens = np.asarray(input_lengths).astype(np.int64)
    # valid[p, c] = ((p%32)*128 + c) < len(utterance p//32)
    tvals = (np.arange(P) % RPU)[:, None] * NT + np.arange(NT)[None, :]
    in_maps = []
    for core in range(NCORES):
        lp_c = log_probs[core * BLOC : (core + 1) * BLOC].reshape(F, V)
        lens_c = lens[core * BLOC : (core + 1) * BLOC]
        vmask = (tvals < lens_c[np.arange(P) // RPU][:, None]).astype(np.float32)
        in_maps.append({"lp": lp_c, "valid": np.ascontiguousarray(vmask)})
    return in_maps


def _grid_to_bt(arr):
    # arr [P, NT]: value for frame t = (p%32)*128 + c of utterance p//32
    return arr.reshape(BLOC, RPU * NT)


def kernel(log_probs, input_lengths, **_kw):
    if "nc" not in _CACHE:
        _CACHE["nc"] = _build_program()
    nc = _CACHE["nc"]
    in_maps = _host_inputs(log_probs, input_lengths)
    res = run_bass_kernel_spmd(nc, in_maps, core_ids=list(range(NCORES)))
    preds = np.empty((B, T), dtype=np.int32)
    keep = np.empty((B, T), dtype=bool)
    max_logp = np.empty((B, T), dtype=np.float32)
    for c, r in enumerate(res.results):
        sl = slice(c * BLOC, (c + 1) * BLOC)
        preds[sl] = _grid_to_bt(r["preds"])
        keep[sl] = _grid_to_bt(r["keep"]).astype(bool)
        max_logp[sl] = _grid_to_bt(r["maxlp"])
    return preds, keep, max_logp



# revision 9
# speedup vs baseline: 1.0095x; 1.0095x over previous
"""CTC greedy decode kernel for Trainium2 (Bass/Tile), 8-core data-parallel.

Problem: log_probs [32, 4096, 1025] f32, input_lengths [32] i64 ->
  preds    [32, 4096] int32  (per-frame argmax)
  keep     [32, 4096] bool   (non-blank & != prev & t < len)
  max_logp [32, 4096] f32    (value at argmax)

Sharding: batch dim across 8 cores (4 utterances each). Per core:
16384 frames x 1025 vocab as a [128, 128] grid of frame-tiles with
frame f = p*128 + c; each utterance owns 32 partition rows and time
runs along the free dim (t = (p%32)*128 + c), so the CTC "previous
frame" is a shifted-AP compare. Per column: DVE tensor_tensor_scan
prefix-max + ACT Sign/accumulate extracts the exact argmax index
(first-occurrence) and max in one pass each, leaving the 67MB/core
HBM stream (~186.6us at the 360GB/s model bandwidth) as the critical
path.

Tail structure (the part that distinguishes this from the ~196.1us
three-store version):
- Outputs are packed field-major into two f32 DRAM tensors:
    outA [128, 324]: cols 0..107   (preds | keep | mlp blocks of 108)
    outB [128, 64] : cols 108..127 (preds 0:20 | keep 20:40 | mlp
                     40:60, col-0 keep at 60)
  outA goes out as one HWDGE DMA issued right after the last input
  load, so its transfer overlaps the tail compute; outB goes out via a
  SWDGE scatter-add (outputs are pre-zeroed) whose descriptor-gen runs
  mid-tail on Pool and whose trigger_dma fires with only a ~60ns Pool
  dispatch - no 625ns HWDGE + 650ns DGE on the critical path.
- Load order: 4-col groups 0..107, then per-column 108..112, 127,
  113..124, col 125 split in two chained pieces scanned on POOL
  (frees DVE), col 126 last in two pieces handled by DVE
  InstMax/InstMaxIndex + a min/blend combine - the last column's index
  needs no ACT pass and no scan, so the post-stream chain is just the
  two piece passes + a few [128,1..3] DVE ops + the triggered 182ns
  scatter.
- The full-width collapse masks for cols 0..107 are interleaved into
  the per-column phase (DVE slack); cols 108..124 masks run on Pool
  parallel to the DVE piece ops; cols 125..127 masks are three fused
  [128,3] DVE ops at the end.
- valid mask is built on-device from an iota + per-partition length
  threshold (56ns DMA instead of a 182ns [128,128] load).

Cost model (TimelineSim): 193877 ns vs 196110 baseline; all outputs
bit-exact vs the reference.
"""

import numpy as np

import concourse.bacc as bacc
import concourse.mybir as mybir
from concourse.tile import TileContext
from concourse.bass_utils import run_bass_kernel_spmd

B, T, V = 32, 4096, 1025
BLANK = 1024
NCORES = 8
BLOC = B // NCORES        # utterances per core
F = BLOC * T              # frames per core
P = 128                   # partitions
NT = F // P               # grid columns (128)
RPU = P // BLOC           # partition rows per utterance (32)
NGRP = 27                 # 4-col groups covering cols 0..107
NA = 108                  # columns in the A (early) store
X1 = 350                  # col-126 piece-1 size

SIGN = mybir.ActivationFunctionType.Sign
MAX = mybir.AluOpType.max
NE = mybir.AluOpType.not_equal
MULT = mybir.AluOpType.mult
ADD = mybir.AluOpType.add
MIN = mybir.AluOpType.min
SUB = mybir.AluOpType.subtract
LT = mybir.AluOpType.is_lt

f32 = mybir.dt.float32
i32 = mybir.dt.int32
i16 = mybir.dt.int16
u32 = mybir.dt.uint32

_CACHE = {}


def _build_program():
    nc = bacc.Bacc(None, target_bir_lowering=False)
    lp = nc.dram_tensor("lp", [F, V], f32, kind="ExternalInput")
    lensadj = nc.dram_tensor("lensadj", [P, 1], f32, kind="ExternalInput")
    sidx = nc.dram_tensor("sidx", [P, 8], i16, kind="ExternalInput")
    outA = nc.dram_tensor("outA", [P, 3 * NA], f32, kind="ExternalOutput")
    outB = nc.dram_tensor("outB", [P, 64], f32, kind="ExternalOutput")
    lp_r = lp.rearrange("(p n) v -> p n v", n=NT)
    dma_sem = nc.alloc_semaphore("swdge_dma")

    with TileContext(nc) as tc:
        with (
            tc.tile_pool(name="loads", bufs=4) as loads,
            tc.tile_pool(name="tails", bufs=12) as tails,
            tc.tile_pool(name="pms", bufs=3) as pms,
            tc.tile_pool(name="pmts", bufs=6) as pmts,
            tc.tile_pool(name="sgs", bufs=4) as sgs,
            tc.tile_pool(name="persist", bufs=1) as pp,
        ):
            first = loads.tile([P, 4, V], f32, tag="big")
            nc.sync.dma_start(out=first[:], in_=lp_r[:, 0:4, :])

            pA = pp.tile([P, 3 * NA], f32)     # preds|keep|mlp, cols 0..107
            p2f = pp.tile([P, 1, 64], f32)     # late cols, scatter payload
            validt = pp.tile([P, NT], f32)
            wrap = pp.tile([P, 1], f32)
            k1t = pp.tile([P, NA], f32)
            kpt = pp.tile([P, NA], f32)
            kbt = pp.tile([P, 17], f32)    # late kp scratch (cols 108..124)
            kb1t = pp.tile([P, 17], f32)   # late k1 scratch
            kc3 = pp.tile([P, 3], f32)     # cols 125..127 kp scratch
            kd3 = pp.tile([P, 3], f32)     # cols 125..127 k1 scratch
            idxs = pp.tile([P, 8], i16)
            mx1 = pp.tile([P, 8], f32)
            mx2 = pp.tile([P, 8], f32)
            ix1 = pp.tile([P, 8], u32)
            ix2 = pp.tile([P, 8], u32)
            i1f = pp.tile([P, 1], f32)
            i2f = pp.tile([P, 1], f32)
            cnd = pp.tile([P, 1], f32)
            uu = pp.tile([P, 1], f32)
            kc1 = pp.tile([P, 1], f32)
            lenst = pp.tile([P, 1], f32)
            iotat = pp.tile([P, NT], i32)

            nc.sync.dma_start(out=lenst[:], in_=lensadj[:])
            # valid[p,c] = (p*128 + c) < len[p//32] + (p//32)*4096
            nc.gpsimd.iota(
                iotat[:], pattern=[[1, NT]], base=0, channel_multiplier=NT
            )
            nc.gpsimd.tensor_scalar(
                out=validt[:], in0=iotat[:],
                scalar1=lenst[:], scalar2=None, op0=LT,
            )
            # p2f must be fully written before the scatter fires; zero the
            # pad (and everything else) up front.
            nc.gpsimd.memset(p2f[:], 0.0)
            # col-0 keep lives in the scatter payload; its pA slot is never
            # written but is covered by the A store, so zero it once.
            nc.gpsimd.memset(pA[:, NA : NA + 1], 0.0)
            # scatter row map: token i -> row i, wrapped in 16
            # partitions and replicated for all 8 GPSIMD cores (the HW
            # reads each core's own 16-partition group; sims read only
            # partitions 0..15). Host-built constant - iota cannot express
            # p%16 and its channel indexing differs between sim and HW.
            nc.sync.dma_start(out=idxs[:], in_=sidx[:])
            prep_ins = nc.gpsimd.dma_scatter_add(
                outB[:, :], p2f[:, :, :], idxs[:, :],
                128, 128, 64,
                prepare_only=True, sem=dma_sem,
            ).ins
            # the wrap DMAs skip utterance-start partitions, so all four
            # sentinels can be set once up front
            for u_ in range(BLOC):
                nc.gpsimd.memset(wrap[u_ * RPU : u_ * RPU + 1, :], -1.0)

            def pred_slot(c):
                if c < NA:
                    return pA[:, c : c + 1]
                return p2f[:, 0, c - NA : c - NA + 1]

            def tile_pass(src2d, pm2d, col):
                nc.vector.tensor_tensor_scan(
                    out=pm2d, data0=src2d, data1=src2d,
                    initial=-3.0e38, op0=MAX, op1=MAX,
                )
                sg = sgs.tile([P, V], f32, tag="sg")
                nc.scalar.activation(
                    out=sg[:], in_=pm2d, func=SIGN,
                    bias=pm2d[:, V - 1 : V], scale=-1.0,
                    accum_out=pred_slot(col),
                )

            def percol(c):
                bt = tails.tile([P, 1, V], f32, tag="tail")
                nc.sync.dma_start(out=bt[:], in_=lp_r[:, c : c + 1, :])
                pmt = pmts.tile([P, 1, V], f32, tag="pmt")
                tile_pass(bt[:, 0, :], pmt[:, 0, :], c)
                nc.gpsimd.tensor_copy(
                    out=p2f[:, 0, 40 + c - NA : 41 + c - NA],
                    in_=pmt[:, 0, V - 1 : V],
                )

            # ---- group phase: cols 0..107 ----
            for g in range(NGRP):
                c0 = 4 * g
                if g == 0:
                    big = first
                else:
                    big = loads.tile([P, 4, V], f32, tag="big")
                    nc.sync.dma_start(out=big[:], in_=lp_r[:, c0 : c0 + 4, :])
                pm = pms.tile([P, 4, V], f32, tag="pm")
                for i in range(4):
                    tile_pass(big[:, i, :], pm[:, i, :], c0 + i)
                nc.gpsimd.tensor_copy(
                    out=pA[:, 2 * NA + c0 : 2 * NA + c0 + 4],
                    in_=pm[:, :, V - 1],
                )

            # ---- per-column phase ----
            for c in (108, 109, 110, 111, 112, 127):
                percol(c)

            def percol_pieces(c, cuts):
                # chained scan pieces: tapered sizes keep the in-order DVE
                # queue tracking land+900 so it is free right when the
                # col-126 piece data becomes visible (a full-column scan
                # would still be running ~570ns after the stream ends)
                bt = tails.tile([P, 1, V], f32, tag="tail")
                for ci in range(len(cuts) - 1):
                    a, b = cuts[ci], cuts[ci + 1]
                    nc.sync.dma_start(
                        out=bt[:, :, a:b], in_=lp_r[:, c : c + 1, a:b]
                    )
                pmt = pmts.tile([P, 1, V], f32, tag="pmt")
                for ci in range(len(cuts) - 1):
                    a, b = cuts[ci], cuts[ci + 1]
                    nc.vector.tensor_tensor_scan(
                        out=pmt[:, 0, a:b], data0=bt[:, 0, a:b],
                        data1=bt[:, 0, a:b],
                        initial=(
                            -3.0e38 if ci == 0 else pmt[:, 0, a - 1 : a]
                        ),
                        op0=MAX, op1=MAX,
                    )
                sg = sgs.tile([P, V], f32, tag="sg")
                nc.scalar.activation(
                    out=sg[:], in_=pmt[:, 0, :], func=SIGN,
                    bias=pmt[:, 0, V - 1 : V], scale=-1.0,
                    accum_out=pred_slot(c),
                )
                nc.gpsimd.tensor_copy(
                    out=p2f[:, 0, 40 + c - NA : 41 + c - NA],
                    in_=pmt[:, 0, V - 1 : V],
                )

            # full-width masks for cols 0..107, interleaved between
            # per-column scans so the in-order DVE queue never stalls on a
            # not-yet-satisfied dependency (sign107 / wrap) while loads
            # still need their scans issued.
            for c in range(113, 122):
                percol(c)
                if c == 115:
                    nc.vector.tensor_scalar(
                        out=k1t[:, :], in0=pA[:, 0:NA],
                        scalar1=float(BLANK), scalar2=None, op0=NE,
                    )
                elif c == 116:
                    nc.vector.tensor_tensor(
                        out=k1t[:, :], in0=k1t[:, :], in1=validt[:, 0:NA],
                        op=MULT,
                    )
                elif c == 117:
                    nc.vector.tensor_tensor(
                        out=kpt[:, 1:NA], in0=pA[:, 1:NA],
                        in1=pA[:, 0 : NA - 1], op=NE,
                    )
                elif c == 118:
                    nc.vector.tensor_tensor(
                        out=pA[:, NA + 1 : 2 * NA], in0=kpt[:, 1:NA],
                        in1=k1t[:, 1:NA], op=MULT,
                    )

            # ---- cols 122..125: chained scan pieces on DVE (GPSIMD can't
            # run tensor_tensor_scan on real HW), tapered so the DVE queue
            # tracks land+900 and is free when col-126 piece data arrives.
            for c in (122, 123, 124, 125):
                percol_pieces(c, (0, 342, 684, V))

            # ---- col 126: two pieces, DVE max/max_index ----
            bt126 = tails.tile([P, 1, V], f32, tag="tail")
            nc.sync.dma_start(
                out=bt126[:, :, 0:X1], in_=lp_r[:, 126:127, 0:X1]
            )
            nc.sync.dma_start(
                out=bt126[:, :, X1:V], in_=lp_r[:, 126:127, X1:V]
            )
            # wrap staging: prev of col 0 is idx[p-1, 127]. On SP after the
            # last input load: its idx127 wait is long satisfied, so it
            # neither stalls the in-order load queue (emitted earlier it
            # would) nor poisons Pool's engine-tick stream (issued from
            # Pool, its DMA-queue latency would gate every later Pool-tick
            # wait). Four transfers that skip the utterance-start
            # partitions, so the -1 sentinels set at startup survive.
            # Col-0's keep goes out via the scatter (slot 60), so the A
            # store below doesn't depend on it.
            for u_ in range(BLOC):
                lo = u_ * RPU + 1
                hi = (u_ + 1) * RPU
                nc.sync.dma_start(
                    out=wrap[lo:hi, :], in_=p2f[lo - 1 : hi - 1, 0, 19:20]
                )
            # A store: issued after the last input load so its transfer
            # queues behind the stream and overlaps the tail compute.
            nc.sync.dma_start(out=outA[:], in_=pA[:])

            nc.vector.max(out=mx1[:], in_=bt126[:, 0, 0:X1])
            nc.vector.max_index(
                out=ix1[:], in_max=mx1[:], in_values=bt126[:, 0, 0:X1]
            )
            nc.vector.max(out=mx2[:], in_=bt126[:, 0, X1:V])
            nc.vector.max_index(
                out=ix2[:], in_max=mx2[:], in_values=bt126[:, 0, X1:V]
            )

            # Pool: i1f cast, then masks for cols 108..124 only (they need
            # idx124 but NOT idx125/126) so the Pool queue reaches the
            # scatter prep early and its ~1us desc-gen hides under the DVE
            # piece ops instead of sitting on the tail. GPSIMD has no
            # compare ops, so != is computed as a squared difference
            # (nonzero iff different - the host reads keep as != 0) and
            # != BLANK as pred < 1024 (preds are in [0, 1024]).
            nc.gpsimd.tensor_copy(out=i1f[:], in_=ix1[:, 0:1])
            nc.gpsimd.tensor_tensor(
                out=kbt[:, 0:1], in0=p2f[:, 0, 0:1], in1=pA[:, NA - 1 : NA],
                op=SUB,
            )
            nc.gpsimd.tensor_tensor(
                out=kbt[:, 1:17], in0=p2f[:, 0, 1:17], in1=p2f[:, 0, 0:16],
                op=SUB,
            )
            nc.gpsimd.tensor_tensor(
                out=kbt[:, :], in0=kbt[:, :], in1=kbt[:, :], op=MULT,
            )
            nc.gpsimd.tensor_scalar(
                out=kb1t[:, :], in0=p2f[:, 0, 0:17],
                scalar1=float(BLANK), scalar2=None, op0=LT,
            )
            nc.gpsimd.tensor_tensor(
                out=kb1t[:, :], in0=kb1t[:, :], in1=validt[:, NA : NA + 17],
                op=MULT,
            )
            nc.gpsimd.tensor_tensor(
                out=p2f[:, 0, 20:37], in0=kbt[:, :], in1=kb1t[:, :], op=MULT,
            )
            # col-0 keep (needs the wrap): host reads it from B slot 60
            nc.gpsimd.tensor_tensor(
                out=kc1[:], in0=pA[:, 0:1], in1=wrap[:], op=SUB,
            )
            nc.gpsimd.tensor_tensor(
                out=kc1[:], in0=kc1[:], in1=kc1[:], op=MULT,
            )
            nc.gpsimd.tensor_tensor(
                out=p2f[:, 0, 60:61], in0=kc1[:], in1=k1t[:, 0:1], op=MULT,
            )

            # DVE: combine pieces -> idx126, then fused cols-125..127 masks
            nc.vector.tensor_scalar(
                out=i2f[:], in0=ix2[:, 0:1],
                scalar1=float(X1), scalar2=None, op0=ADD,
            )
            nc.vector.tensor_tensor(
                out=cnd[:], in0=mx1[:, 0:1], in1=mx2[:, 0:1], op=LT,
            )
            nc.vector.tensor_scalar(
                out=uu[:], in0=cnd[:],
                scalar1=2000.0, scalar2=i1f[:], op0=MULT, op1=ADD,
            )
            nc.vector.tensor_tensor(
                out=p2f[:, 0, 18:19], in0=uu[:], in1=i2f[:], op=MIN,
            )
            nc.vector.tensor_tensor(
                out=p2f[:, 0, 58:59], in0=mx1[:, 0:1], in1=mx2[:, 0:1], op=MAX,
            )
            nc.vector.tensor_tensor(
                out=kc3[:, :], in0=p2f[:, 0, 17:20], in1=p2f[:, 0, 16:19],
                op=NE,
            )
            nc.vector.tensor_scalar(
                out=kd3[:, :], in0=p2f[:, 0, 17:20],
                scalar1=float(BLANK), scalar2=None, op0=NE,
            )
            nc.vector.tensor_tensor(
                out=kd3[:, :], in0=kd3[:, :], in1=validt[:, 125:128], op=MULT,
            )
            nc.vector.tensor_tensor(
                out=p2f[:, 0, 37:40], in0=kc3[:, :], in1=kd3[:, :], op=MULT,
            )

            nc.gpsimd.trigger_dma(count=None)

    # Tile's end-of-program wait targets the prep's DMASW lane sem, but a
    # prepare_only scatter signals completion via the user sem baked into
    # the descriptor (sem=) - nothing ever bumps the lane sem. Retarget
    # waits on the prep's lane at the user sem so every executor (timeline
    # sim, interp, device) sees the same, satisfiable condition.
    from concourse.tile_scheduler import PROC_NAMES

    lane = PROC_NAMES[prep_ins.bass_scheduled_proc]
    dmasw_waits = 0
    for blk in nc.m.functions[0].blocks:
        for ins in blk.instructions:
            si = ins.sync_info
            if not si:
                continue
            for w in si.on_wait:
                if w.ant_name and w.ant_name.startswith(lane + "_"):
                    w.id = dma_sem.num
                    w.ant_name = dma_sem.name
                    dmasw_waits += 1
    assert dmasw_waits == 1, (lane, dmasw_waits)
    nc.compile()
    return nc


def _host_inputs(log_probs, input_lengths):
    log_probs = np.ascontiguousarray(np.asarray(log_probs, dtype=np.float32))
    lens = np.asarray(input_lengths).astype(np.int64)
    us = np.arange(P) // RPU
    in_maps = []
    for core in range(NCORES):
        lp_c = log_probs[core * BLOC : (core + 1) * BLOC].reshape(F, V)
        lens_c = lens[core * BLOC : (core + 1) * BLOC]
        lensadj = (lens_c[us] + us * T).astype(np.float32)[:, None]
        sidx = (
            16 * np.arange(8)[None, :] + (np.arange(P) % 16)[:, None]
        ).astype(np.int16)
        in_maps.append(
            {"lp": lp_c, "lensadj": np.ascontiguousarray(lensadj),
             "sidx": np.ascontiguousarray(sidx)}
        )
    return in_maps


def _grid_to_bt(arr):
    # arr [P, NT]: value for frame t = (p%32)*128 + c of utterance p//32
    return arr.reshape(BLOC, RPU * NT)


def kernel(log_probs, input_lengths, **_kw):
    if "nc" not in _CACHE:
        _CACHE["nc"] = _build_program()
    nc = _CACHE["nc"]
    in_maps = _host_inputs(log_probs, input_lengths)
    res = run_bass_kernel_spmd(nc, in_maps, core_ids=list(range(NCORES)))
    preds = np.empty((B, T), dtype=np.int32)
    keep = np.empty((B, T), dtype=bool)
    max_logp = np.empty((B, T), dtype=np.float32)
    for c, r in enumerate(res.results):
        a, b = r["outA"], r["outB"]
        pg = np.concatenate([a[:, 0:NA], b[:, 0:20]], axis=1)
        kg = np.concatenate([a[:, NA : 2 * NA], b[:, 20:40]], axis=1)
        kg[:, 0] = b[:, 60]
        mg = np.concatenate([a[:, 2 * NA : 3 * NA], b[:, 40:60]], axis=1)
        sl = slice(c * BLOC, (c + 1) * BLOC)
        preds[sl] = _grid_to_bt(pg).astype(np.int32)
        keep[sl] = _grid_to_bt(kg).astype(bool)
        max_logp[sl] = _grid_to_bt(mg)
    return preds, keep, max_logp


# revision 13
# speedup vs baseline: 1.0106x; 1.0011x over previous
"""CTC greedy decode kernel for Trainium2 (Bass/Tile), 8-core data-parallel.

Problem: log_probs [32, 4096, 1025] f32, input_lengths [32] i64 ->
  preds    [32, 4096] int32  (per-frame argmax)
  keep     [32, 4096] bool   (non-blank & != prev & t < len)
  max_logp [32, 4096] f32    (value at argmax)

Sharding: batch dim across 8 cores (4 utterances each). Per core:
16384 frames x 1025 vocab as a [128, 128] grid of frame-tiles with
frame f = p*128 + c; each utterance owns 32 partition rows and time
runs along the free dim (t = (p%32)*128 + c), so the CTC "previous
frame" is a shifted-AP compare. Per column: DVE tensor_tensor_scan
prefix-max + ACT Sign/accumulate extracts the exact argmax index
(first-occurrence) and max in one pass each, leaving the 67MB/core
HBM stream (~186.6us at the 360GB/s model bandwidth) as the critical
path.

Tail structure (the part that distinguishes this from the ~196.1us
three-store version):
- Outputs are packed field-major into two f32 DRAM tensors:
    outA [128, 324]: cols 0..107   (preds | keep | mlp blocks of 108)
    outB [128, 64] : cols 108..127 (preds 0:20 | keep 20:40 | mlp
                     40:60, col-0 keep at 60)
  outA goes out as one HWDGE DMA issued right after the last input
  load, so its transfer overlaps the tail compute; outB goes out via a
  SWDGE scatter-add (outputs are pre-zeroed) whose descriptor-gen runs
  mid-tail on Pool and whose trigger_dma fires with only a ~60ns Pool
  dispatch - no 625ns HWDGE + 650ns DGE on the critical path.
- Load order: 4-col groups 0..107, then per-column 108..112, 127,
  113..124, col 125 split in two chained pieces scanned on POOL
  (frees DVE), col 126 last in two pieces handled by DVE
  InstMax/InstMaxIndex + a min/blend combine - the last column's index
  needs no ACT pass and no scan, so the post-stream chain is just the
  two piece passes + a few [128,1..3] DVE ops + the triggered 182ns
  scatter.
- The full-width collapse masks for cols 0..107 are interleaved into
  the per-column phase (DVE slack); cols 108..124 masks run on Pool
  parallel to the DVE piece ops; cols 125..127 masks are three fused
  [128,3] DVE ops at the end.
- valid mask is built on-device from an iota + per-partition length
  threshold (56ns DMA instead of a 182ns [128,128] load).

Cost model (TimelineSim): 193877 ns vs 196110 baseline; all outputs
bit-exact vs the reference.
"""

import numpy as np

import concourse.bacc as bacc
import concourse.mybir as mybir
from concourse.tile import TileContext
from concourse.bass_utils import run_bass_kernel_spmd

B, T, V = 32, 4096, 1025
BLANK = 1024
NCORES = 8
BLOC = B // NCORES        # utterances per core
F = BLOC * T              # frames per core
P = 128                   # partitions
NT = F // P               # grid columns (128)
RPU = P // BLOC           # partition rows per utterance (32)
NGRP = 27                 # 4-col groups covering cols 0..107
NA = 108                  # columns in the A (early) store
X1 = 350                  # col-126 piece-1 size

SIGN = mybir.ActivationFunctionType.Sign
MAX = mybir.AluOpType.max
NE = mybir.AluOpType.not_equal
MULT = mybir.AluOpType.mult
ADD = mybir.AluOpType.add
MIN = mybir.AluOpType.min
SUB = mybir.AluOpType.subtract
LT = mybir.AluOpType.is_lt

f32 = mybir.dt.float32
i32 = mybir.dt.int32
i16 = mybir.dt.int16
u32 = mybir.dt.uint32

_CACHE = {}


def _build_program():
    nc = bacc.Bacc(None, target_bir_lowering=False)
    lp = nc.dram_tensor("lp", [F, V], f32, kind="ExternalInput")
    lensadj = nc.dram_tensor("lensadj", [P, 1], f32, kind="ExternalInput")
    sidx = nc.dram_tensor("sidx", [P, 8], i16, kind="ExternalInput")
    outA = nc.dram_tensor("outA", [P, 3 * NA], f32, kind="ExternalOutput")
    outB = nc.dram_tensor("outB", [P, 64], f32, kind="ExternalOutput")
    lp_r = lp.rearrange("(p n) v -> p n v", n=NT)
    dma_sem = nc.alloc_semaphore("swdge_dma")

    with TileContext(nc) as tc:
        with (
            tc.tile_pool(name="loads", bufs=4) as loads,
            tc.tile_pool(name="tails", bufs=12) as tails,
            tc.tile_pool(name="pms", bufs=3) as pms,
            tc.tile_pool(name="pmts", bufs=6) as pmts,
            tc.tile_pool(name="sgs", bufs=4) as sgs,
            tc.tile_pool(name="persist", bufs=1) as pp,
        ):
            first = loads.tile([P, 4, V], f32, tag="big")
            nc.sync.dma_start(out=first[:], in_=lp_r[:, 0:4, :])

            pA = pp.tile([P, 3 * NA], f32)     # preds|keep|mlp, cols 0..107
            p2f = pp.tile([P, 1, 64], f32)     # late cols, scatter payload
            validt = pp.tile([P, NT], f32)
            wrap = pp.tile([P, 1], f32)
            k1t = pp.tile([P, NA], f32)
            kpt = pp.tile([P, NA], f32)
            kbt = pp.tile([P, 17], f32)    # late kp scratch (cols 108..124)
            kb1t = pp.tile([P, 17], f32)   # late k1 scratch
            kc3 = pp.tile([P, 3], f32)     # cols 125..127 kp scratch
            kd3 = pp.tile([P, 3], f32)     # cols 125..127 k1 scratch
            idxs = pp.tile([P, 8], i16)
            mx1 = pp.tile([P, 8], f32)
            mx2 = pp.tile([P, 8], f32)
            ix1 = pp.tile([P, 8], u32)
            ix2 = pp.tile([P, 8], u32)
            i1f = pp.tile([P, 1], f32)
            i2f = pp.tile([P, 1], f32)
            cnd = pp.tile([P, 1], f32)
            uu = pp.tile([P, 1], f32)
            kc1 = pp.tile([P, 1], f32)
            lenst = pp.tile([P, 1], f32)
            iotat = pp.tile([P, NT], i32)

            nc.sync.dma_start(out=lenst[:], in_=lensadj[:])
            # valid[p,c] = (p*128 + c) < len[p//32] + (p//32)*4096
            nc.gpsimd.iota(
                iotat[:], pattern=[[1, NT]], base=0, channel_multiplier=NT
            )
            nc.gpsimd.tensor_scalar(
                out=validt[:], in0=iotat[:],
                scalar1=lenst[:], scalar2=None, op0=LT,
            )
            # p2f must be fully written before the scatter fires; zero the
            # pad (and everything else) up front.
            nc.gpsimd.memset(p2f[:], 0.0)
            # col-0 keep lives in the scatter payload; its pA slot is never
            # written but is covered by the A store, so zero it once.
            nc.gpsimd.memset(pA[:, NA : NA + 1], 0.0)
            # the wrap DMAs skip utterance-start partitions, so all four
            # sentinels can be set once up front
            for u_ in range(BLOC):
                nc.gpsimd.memset(wrap[u_ * RPU : u_ * RPU + 1, :], -1.0)

            def pred_slot(c):
                if c < NA:
                    return pA[:, c : c + 1]
                return p2f[:, 0, c - NA : c - NA + 1]

            def tile_pass(src2d, pm2d, col):
                nc.vector.tensor_tensor_scan(
                    out=pm2d, data0=src2d, data1=src2d,
                    initial=-3.0e38, op0=MAX, op1=MAX,
                )
                sg = sgs.tile([P, V], f32, tag="sg")
                nc.scalar.activation(
                    out=sg[:], in_=pm2d, func=SIGN,
                    bias=pm2d[:, V - 1 : V], scale=-1.0,
                    accum_out=pred_slot(col),
                )

            def percol(c):
                bt = tails.tile([P, 1, V], f32, tag="tail")
                nc.sync.dma_start(out=bt[:], in_=lp_r[:, c : c + 1, :])
                pmt = pmts.tile([P, 1, V], f32, tag="pmt")
                tile_pass(bt[:, 0, :], pmt[:, 0, :], c)
                nc.gpsimd.tensor_copy(
                    out=p2f[:, 0, 40 + c - NA : 41 + c - NA],
                    in_=pmt[:, 0, V - 1 : V],
                )

            # ---- group phase: cols 0..107 ----
            for g in range(NGRP):
                c0 = 4 * g
                if g == 0:
                    big = first
                else:
                    big = loads.tile([P, 4, V], f32, tag="big")
                    nc.sync.dma_start(out=big[:], in_=lp_r[:, c0 : c0 + 4, :])
                pm = pms.tile([P, 4, V], f32, tag="pm")
                for i in range(4):
                    tile_pass(big[:, i, :], pm[:, i, :], c0 + i)
                nc.gpsimd.tensor_copy(
                    out=pA[:, 2 * NA + c0 : 2 * NA + c0 + 4],
                    in_=pm[:, :, V - 1],
                )

            # ---- per-column phase ----
            for c in (108, 109, 110, 111, 112, 127):
                percol(c)

            def percol_pieces(c, cuts):
                # chained scan pieces: tapered sizes keep the in-order DVE
                # queue tracking land+900 so it is free right when the
                # col-126 piece data becomes visible (a full-column scan
                # would still be running ~570ns after the stream ends)
                bt = tails.tile([P, 1, V], f32, tag="tail")
                for ci in range(len(cuts) - 1):
                    a, b = cuts[ci], cuts[ci + 1]
                    nc.sync.dma_start(
                        out=bt[:, :, a:b], in_=lp_r[:, c : c + 1, a:b]
                    )
                pmt = pmts.tile([P, 1, V], f32, tag="pmt")
                for ci in range(len(cuts) - 1):
                    a, b = cuts[ci], cuts[ci + 1]
                    nc.vector.tensor_tensor_scan(
                        out=pmt[:, 0, a:b], data0=bt[:, 0, a:b],
                        data1=bt[:, 0, a:b],
                        initial=(
                            -3.0e38 if ci == 0 else pmt[:, 0, a - 1 : a]
                        ),
                        op0=MAX, op1=MAX,
                    )
                sg = sgs.tile([P, V], f32, tag="sg")
                nc.scalar.activation(
                    out=sg[:], in_=pmt[:, 0, :], func=SIGN,
                    bias=pmt[:, 0, V - 1 : V], scale=-1.0,
                    accum_out=pred_slot(c),
                )
                nc.gpsimd.tensor_copy(
                    out=p2f[:, 0, 40 + c - NA : 41 + c - NA],
                    in_=pmt[:, 0, V - 1 : V],
                )

            # full-width masks for cols 0..107, interleaved between
            # per-column scans so the in-order DVE queue never stalls on a
            # not-yet-satisfied dependency (sign107 / wrap) while loads
            # still need their scans issued.
            for c in range(113, 122):
                percol(c)
                if c == 115:
                    nc.vector.tensor_scalar(
                        out=k1t[:, :], in0=pA[:, 0:NA],
                        scalar1=float(BLANK), scalar2=None, op0=NE,
                    )
                elif c == 116:
                    nc.vector.tensor_tensor(
                        out=k1t[:, :], in0=k1t[:, :], in1=validt[:, 0:NA],
                        op=MULT,
                    )
                elif c == 117:
                    nc.vector.tensor_tensor(
                        out=kpt[:, 1:NA], in0=pA[:, 1:NA],
                        in1=pA[:, 0 : NA - 1], op=NE,
                    )
                elif c == 118:
                    nc.vector.tensor_tensor(
                        out=pA[:, NA + 1 : 2 * NA], in0=kpt[:, 1:NA],
                        in1=k1t[:, 1:NA], op=MULT,
                    )

            # ---- cols 122..125: chained scan pieces on DVE (GPSIMD can't
            # run tensor_tensor_scan on real HW), tapered so the DVE queue
            # tracks land+900 and is free when col-126 piece data arrives.
            percol_pieces(122, (0, 512, V))
            percol_pieces(123, (0, 512, V))
            percol_pieces(124, (0, 512, V))
            percol_pieces(125, (0, 512, V))

            # ---- col 126: two pieces, DVE max/max_index ----
            bt126 = tails.tile([P, 1, V], f32, tag="tail")
            nc.sync.dma_start(
                out=bt126[:, :, 0:X1], in_=lp_r[:, 126:127, 0:X1]
            )
            nc.sync.dma_start(
                out=bt126[:, :, X1:V], in_=lp_r[:, 126:127, X1:V]
            )
            # scatter row map: token i -> row i, wrapped in 16
            # partitions and replicated for all 8 GPSIMD cores (the HW
            # reads each core's own 16-partition group; sims read only
            # partitions 0..15). Host-built constant - iota cannot express
            # p%16 and its channel indexing differs between sim and HW.
            # Loaded after the last input so its 56ns transfer overlaps
            # the tail; the prep reads it only at desc-gen time mid-tail.
            nc.sync.dma_start(out=idxs[:], in_=sidx[:])
            prep_ins = nc.gpsimd.dma_scatter_add(
                outB[:, :], p2f[:, :, :], idxs[:, :],
                128, 128, 64,
                prepare_only=True, sem=dma_sem,
            ).ins
            # wrap staging: prev of col 0 is idx[p-1, 127]. On SP after the
            # last input load: its idx127 wait is long satisfied, so it
            # neither stalls the in-order load queue (emitted earlier it
            # would) nor poisons Pool's engine-tick stream (issued from
            # Pool, its DMA-queue latency would gate every later Pool-tick
            # wait). Four transfers that skip the utterance-start
            # partitions, so the -1 sentinels set at startup survive.
            # Col-0's keep goes out via the scatter (slot 60), so the A
            # store below doesn't depend on it.
            for u_ in range(BLOC):
                lo = u_ * RPU + 1
                hi = (u_ + 1) * RPU
                nc.sync.dma_start(
                    out=wrap[lo:hi, :], in_=p2f[lo - 1 : hi - 1, 0, 19:20]
                )
            # A store: issued after the last input load so its transfer
            # queues behind the stream and overlaps the tail compute.
            nc.sync.dma_start(out=outA[:], in_=pA[:])

            nc.vector.max(out=mx1[:], in_=bt126[:, 0, 0:X1])
            nc.vector.max_index(
                out=ix1[:], in_max=mx1[:], in_values=bt126[:, 0, 0:X1]
            )
            nc.vector.max(out=mx2[:], in_=bt126[:, 0, X1:V])
            nc.vector.max_index(
                out=ix2[:], in_max=mx2[:], in_values=bt126[:, 0, X1:V]
            )

            # Pool: i1f cast, then masks for cols 108..124 only (they need
            # idx124 but NOT idx125/126) so the Pool queue reaches the
            # scatter prep early and its ~1us desc-gen hides under the DVE
            # piece ops instead of sitting on the tail. GPSIMD has no
            # compare ops, so != is computed as a squared difference
            # (nonzero iff different - the host reads keep as != 0) and
            # != BLANK as pred < 1024 (preds are in [0, 1024]).
            nc.gpsimd.tensor_copy(out=i1f[:], in_=ix1[:, 0:1])
            nc.gpsimd.tensor_tensor(
                out=kbt[:, 0:1], in0=p2f[:, 0, 0:1], in1=pA[:, NA - 1 : NA],
                op=SUB,
            )
            nc.gpsimd.tensor_tensor(
                out=kbt[:, 1:17], in0=p2f[:, 0, 1:17], in1=p2f[:, 0, 0:16],
                op=SUB,
            )
            nc.gpsimd.tensor_tensor(
                out=kbt[:, :], in0=kbt[:, :], in1=kbt[:, :], op=MULT,
            )
            nc.gpsimd.tensor_scalar(
                out=kb1t[:, :], in0=p2f[:, 0, 0:17],
                scalar1=float(BLANK), scalar2=None, op0=LT,
            )
            nc.gpsimd.tensor_tensor(
                out=kb1t[:, :], in0=kb1t[:, :], in1=validt[:, NA : NA + 17],
                op=MULT,
            )
            nc.gpsimd.tensor_tensor(
                out=p2f[:, 0, 20:37], in0=kbt[:, :], in1=kb1t[:, :], op=MULT,
            )
            # col-0 keep (needs the wrap): host reads it from B slot 60
            nc.gpsimd.tensor_tensor(
                out=kc1[:], in0=pA[:, 0:1], in1=wrap[:], op=SUB,
            )
            nc.gpsimd.tensor_tensor(
                out=kc1[:], in0=kc1[:], in1=kc1[:], op=MULT,
            )
            nc.gpsimd.tensor_tensor(
                out=p2f[:, 0, 60:61], in0=kc1[:], in1=k1t[:, 0:1], op=MULT,
            )

            # DVE: combine pieces -> idx126, then fused cols-125..127 masks
            nc.vector.tensor_scalar(
                out=i2f[:], in0=ix2[:, 0:1],
                scalar1=float(X1), scalar2=None, op0=ADD,
            )
            nc.vector.tensor_tensor(
                out=cnd[:], in0=mx1[:, 0:1], in1=mx2[:, 0:1], op=LT,
            )
            nc.vector.tensor_scalar(
                out=uu[:], in0=cnd[:],
                scalar1=2000.0, scalar2=i1f[:], op0=MULT, op1=ADD,
            )
            nc.vector.tensor_tensor(
                out=p2f[:, 0, 18:19], in0=uu[:], in1=i2f[:], op=MIN,
            )
            nc.vector.tensor_tensor(
                out=p2f[:, 0, 58:59], in0=mx1[:, 0:1], in1=mx2[:, 0:1], op=MAX,
            )
            nc.vector.tensor_tensor(
                out=kc3[:, :], in0=p2f[:, 0, 17:20], in1=p2f[:, 0, 16:19],
                op=NE,
            )
            nc.vector.tensor_scalar(
                out=kd3[:, :], in0=p2f[:, 0, 17:20],
                scalar1=float(BLANK), scalar2=None, op0=NE,
            )
            nc.vector.tensor_tensor(
                out=kd3[:, :], in0=kd3[:, :], in1=validt[:, 125:128], op=MULT,
            )
            nc.vector.tensor_tensor(
                out=p2f[:, 0, 37:40], in0=kc3[:, :], in1=kd3[:, :], op=MULT,
            )

            nc.gpsimd.trigger_dma(count=None)

    # Tile's end-of-program wait targets the prep's DMASW lane sem, but a
    # prepare_only scatter signals completion via the user sem baked into
    # the descriptor (sem=) - nothing ever bumps the lane sem. Retarget
    # waits on the prep's lane at the user sem so every executor (timeline
    # sim, interp, device) sees the same, satisfiable condition.
    from concourse.tile_scheduler import PROC_NAMES

    lane = PROC_NAMES[prep_ins.bass_scheduled_proc]
    dmasw_waits = 0
    for blk in nc.m.functions[0].blocks:
        for ins in blk.instructions:
            si = ins.sync_info
            if not si:
                continue
            for w in si.on_wait:
                if w.ant_name and w.ant_name.startswith(lane + "_"):
                    w.id = dma_sem.num
                    w.ant_name = dma_sem.name
                    dmasw_waits += 1
    assert dmasw_waits == 1, (lane, dmasw_waits)
    nc.compile()
    return nc


def _host_inputs(log_probs, input_lengths):
    log_probs = np.ascontiguousarray(np.asarray(log_probs, dtype=np.float32))
    lens = np.asarray(input_lengths).astype(np.int64)
    us = np.arange(P) // RPU
    in_maps = []
    for core in range(NCORES):
        lp_c = log_probs[core * BLOC : (core + 1) * BLOC].reshape(F, V)
        lens_c = lens[core * BLOC : (core + 1) * BLOC]
        lensadj = (lens_c[us] + us * T).astype(np.float32)[:, None]
        sidx = (
            16 * np.arange(8)[None, :] + (np.arange(P) % 16)[:, None]
        ).astype(np.int16)
        in_maps.append(
            {"lp": lp_c, "lensadj": np.ascontiguousarray(lensadj),
             "sidx": np.ascontiguousarray(sidx)}
        )
    return in_maps


def _grid_to_bt(arr):
    # arr [P, NT]: value for frame t = (p%32)*128 + c of utterance p//32
    return arr.reshape(BLOC, RPU * NT)


def kernel(log_probs, input_lengths, **_kw):
    if "nc" not in _CACHE:
        _CACHE["nc"] = _build_program()
    nc = _CACHE["nc"]
    in_maps = _host_inputs(log_probs, input_lengths)
    res = run_bass_kernel_spmd(nc, in_maps, core_ids=list(range(NCORES)))
    preds = np.empty((B, T), dtype=np.int32)
    keep = np.empty((B, T), dtype=bool)
    max_logp = np.empty((B, T), dtype=np.float32)
    for c, r in enumerate(res.results):
        a, b = r["outA"], r["outB"]
        pg = np.concatenate([a[:, 0:NA], b[:, 0:20]], axis=1)
        kg = np.concatenate([a[:, NA : 2 * NA], b[:, 20:40]], axis=1)
        kg[:, 0] = b[:, 60]
        mg = np.concatenate([a[:, 2 * NA : 3 * NA], b[:, 40:60]], axis=1)
        sl = slice(c * BLOC, (c + 1) * BLOC)
        preds[sl] = _grid_to_bt(pg).astype(np.int32)
        keep[sl] = _grid_to_bt(kg).astype(bool)
        max_logp[sl] = _grid_to_bt(mg)
    return preds, keep, max_logp
